# revision 1
# baseline (speedup 1.0000x reference)
"""Causal self-attention (B=2, S=2048, E=1024, H=16, D=64) on 8 TRN2 cores.

Sharding: core c = (batch b = c//4, head-group g = c%4) owns batch b and
heads 4g..4g+3 (a 256-wide slice of the QKV projections / Wo rows).
Each core computes its partial out-projection y_c = attout_c @ Wo_c; the
host sums the 4 partials per batch and adds bo (the tensor-parallel
out-proj all-reduce, done on host since cores are independent).

All device inputs/outputs are host-packed into [128, X] layouts whose
per-partition rows are contiguous in DRAM, so every DMA is 128 large
contiguous descriptors (DMA-issue cost on the sync sequencer would
otherwise dominate the kernel).

Device pipeline (per core), bf16 operands / fp32 PSUM accumulation:
  Q^T, K^T [256, S] via lhsT=W chunk, rhs=xT chunk
  V natural [S, 4*(64+1)] with a ones column per head (softmax denom)
  scores^T [k, q] per head: lhsT=K^T (D=64 contraction, head pairs
  packed in PE row-groups 0-63/64-127), exp on ACT (scale=1/8 folded),
  causal mask multiply on DVE (diagonal blocks only; upper blocks
  skipped entirely)
  attout^T [65, q] PV matmul, row 64 = softmax denominator
  normalize: reciprocal_approx_fast + gpsimd partition_broadcast + TT mul
  y = attoutT_norm.T @ Wo_c, staged in SBUF, DMA'd out in 4 chunks.
"""

import numpy as np

B, S, E, H = 2, 2048, 1024, 16
D = E // H          # 64
NCORES = 8
HPC = 4             # heads per core
HD = HPC * D        # 256 cols per core
KT = E // 128       # 8 contraction tiles for projections
QC = S // 512       # 4 query chunks
NQT = S // 128      # 16 row tiles
VW = HPC * (D + 1)  # 260: V + ones column per head

_prog = None
LAST_RESULTS = None


def _build_program():
    import concourse.mybir as mybir
    import concourse.tile as tile
    from concourse import bacc, library_config

    f32 = mybir.dt.float32
    bf16 = mybir.dt.bfloat16
    Exp = mybir.ActivationFunctionType.Exp
    Identity = mybir.ActivationFunctionType.Identity

    nc = bacc.Bacc(trn_type="TRN2", target_bir_lowering=False, debug=False)

    xT = nc.dram_tensor("xT", [128, QC * KT * 512], bf16, kind="ExternalInput").ap()
    wq = nc.dram_tensor("wq", [128, KT * HD], bf16, kind="ExternalInput").ap()
    wk = nc.dram_tensor("wk", [128, KT * HD], bf16, kind="ExternalInput").ap()
    wv = nc.dram_tensor("wv", [128, KT * HD], bf16, kind="ExternalInput").ap()
    wo = nc.dram_tensor("wo", [128, 2 * E], bf16, kind="ExternalInput").ap()
    bq = nc.dram_tensor("bqc", [128, 2], f32, kind="ExternalInput").ap()
    bk = nc.dram_tensor("bkc", [128, 2], f32, kind="ExternalInput").ap()
    bv = nc.dram_tensor("bvb", [128, HD], bf16, kind="ExternalInput").ap()
    mask = nc.dram_tensor("mask", [128, 4 * 512], bf16, kind="ExternalInput").ap()
    y = nc.dram_tensor("y", [128, NQT * E], f32, kind="ExternalOutput").ap()

    with tile.TileContext(nc) as tc:
        with (
            tc.tile_pool(name="consts", bufs=1) as consts,
            tc.tile_pool(name="exps", bufs=4) as exps,
            tc.tile_pool(name="small", bufs=4) as small,
            tc.tile_pool(name="ps_sc", bufs=3, space="PSUM") as ps_sc,
            tc.tile_pool(name="ps_acc", bufs=2, space="PSUM") as ps_acc,
        ):
            nc.gpsimd.load_library(library_config.attn)
            # ---- constants; DMA order tuned so qc=0 work starts ASAP ----
            xt_sb = consts.tile([128, QC, KT, 512], bf16)
            wq_sb = consts.tile([128, KT, HD], bf16)
            wk_sb = consts.tile([128, KT, HD], bf16)
            wv_sb = consts.tile([128, KT, HD], bf16)
            wo_sb = consts.tile([128, 2, E], bf16)
            mask_sb = consts.tile([128, 4, 512], bf16)
            bq_sb = consts.tile([128, 2], f32)
            bk_sb = consts.tile([128, 2], f32)
            bv_sb = consts.tile([128, HD], bf16)

            def load_xt(qc):
                nc.sync.dma_start(
                    out=xt_sb[:, qc],
                    in_=xT[:, qc * KT * 512 : (qc + 1) * KT * 512].rearrange(
                        "p (kt c) -> p kt c", kt=KT
                    ),
                )

            nc.sync.dma_start(out=wq_sb, in_=wq.rearrange("p (kt c) -> p kt c", kt=KT))
            load_xt(0)
            nc.sync.dma_start(out=wk_sb, in_=wk.rearrange("p (kt c) -> p kt c", kt=KT))
            nc.sync.dma_start(out=wv_sb, in_=wv.rearrange("p (kt c) -> p kt c", kt=KT))
            nc.sync.dma_start(out=bq_sb, in_=bq)
            nc.sync.dma_start(out=bk_sb, in_=bk)
            nc.sync.dma_start(out=bv_sb, in_=bv)
            load_xt(1)
            nc.sync.dma_start(out=mask_sb, in_=mask.rearrange("p (t c) -> p t c", t=4))
            load_xt(2)
            load_xt(3)
            nc.sync.dma_start(out=wo_sb, in_=wo.rearrange("p (kt c) -> p kt c", kt=2))

            # ---- persistent activations ----
            # Q^T/K^T: [128, mt, S]; mt=0 holds cols 0-127 (heads 0,1),
            # mt=1 holds cols 128-255 (heads 2,3).
            qt_sb = consts.tile([128, 2, S], bf16)
            kt_sb = consts.tile([128, 2, S], bf16)
            # V natural: [row-in-tile, rt, 4*(64+1)]; per head h cols
            # h*65..h*65+63 are V, col h*65+64 is ones.
            v_sb = consts.tile([128, NQT, VW], bf16)
            nc.vector.memset(
                v_sb.rearrange("p rt (h c) -> p rt h c", h=HPC)[:, :, :, D : D + 1],
                1.0,
            )
            # normalized attout^T, same layout as qt_sb
            at_sb = consts.tile([128, 2, S], bf16)
            # full output staging: [p, qt, col]
            y_sb = consts.tile([128, NQT, E], f32)

            # ====== fused per-qc loop: projections -> attention -> out ======
            for qc in range(QC):
                # ---- projections for this q-chunk ----
                for w_sb, b_sb, dst in ((wq_sb, bq_sb, qt_sb), (wk_sb, bk_sb, kt_sb)):
                    ps = ps_sc.tile([128, 1024], f32, tag="sc", name=f"ps_qk{qc}")
                    for mt in range(2):
                        o = ps[:, mt * 512 : mt * 512 + 512]
                        for kt in range(KT):
                            nc.tensor.matmul(
                                o,
                                lhsT=w_sb[:, kt, mt * 128 : mt * 128 + 128],
                                rhs=xt_sb[:, qc, kt],
                                start=(kt == 0),
                                stop=(kt == KT - 1),
                            )
                        # PSUM->SBUF copy on DVE with the bias folded in
                        nc.vector.tensor_scalar_add(
                            dst[:, mt, qc * 512 : (qc + 1) * 512],
                            o,
                            b_sb[:, mt : mt + 1],
                        )

                for half in range(2):  # two V psum tiles, 2 row-tiles each
                    ps = ps_sc.tile([128, 1024], f32, tag="sc", name=f"ps_v{qc}_{half}")
                    for j in range(2):
                        rl = half * 2 + j          # row-tile within chunk (0..3)
                        rt = qc * 4 + rl           # global row tile
                        o = ps[:, j * 512 : j * 512 + HD]
                        for kt in range(KT):
                            nc.tensor.matmul(
                                o,
                                lhsT=xt_sb[:, qc, kt, rl * 128 : rl * 128 + 128],
                                rhs=wv_sb[:, kt, :],
                                start=(kt == 0),
                                stop=(kt == KT - 1),
                            )
                        # PSUM->SBUF with bias added (bvb host-broadcast)
                        nc.vector.tensor_add(
                            v_sb[:, rt, :]
                            .rearrange("p (h c) -> p h c", h=HPC)[:, :, 0:D],
                            o.rearrange("p (h c) -> p h c", h=HPC),
                            bv_sb.rearrange("p (h c) -> p h c", h=HPC),
                        )

                # ---- attention for this q-chunk, both head pairs ----
                nkt = 4 * (qc + 1)       # causal: k-tiles 0..nkt-1
                for mt in range(2):      # head pair (2mt, 2mt+1)
                    acc = [
                        ps_acc.tile([128, 512], f32, tag="acc", name=f"acc{mt}{qc}{j}")
                        for j in range(2)
                    ]
                    for kt in range(nkt):
                        t = kt - 4 * qc
                        # diagonal blocks: columns q < 128*t are fully masked
                        # -> narrow QK/exp/mask/PV to the valid range. PV
                        # never touches the dead columns (other kt wrote
                        # them), so no memset is needed.
                        off = 128 * t if t > 0 else 0
                        w = 512 - off
                        ps = ps_sc.tile([128, 1024], f32, tag="sc", name=f"ps_s{kt}")
                        for j in range(2):   # head within pair
                            pb = j * 64
                            nc.tensor.matmul(
                                ps[:, j * 512 + off : j * 512 + 512],
                                lhsT=kt_sb[pb : pb + 64, mt, kt * 128 : kt * 128 + 128],
                                rhs=qt_sb[
                                    pb : pb + 64, mt,
                                    qc * 512 + off : qc * 512 + 512,
                                ],
                                start=True,
                                stop=True,
                            )
                        ex = exps.tile([128, 1024], bf16, tag="ex", name=f"ex{kt}")
                        # scores scale 1/sqrt(D) folded into exp
                        if off == 0:
                            nc.scalar.activation(ex, ps, Exp, scale=0.125)
                        else:
                            for j in range(2):
                                nc.scalar.activation(
                                    ex[:, j * 512 + off : j * 512 + 512],
                                    ps[:, j * 512 + off : j * 512 + 512],
                                    Exp,
                                    scale=0.125,
                                )
                        for j in range(2):
                            exj = ex[:, j * 512 + off : j * 512 + 512]
                            if t >= 0:  # diagonal block: causal mask
                                nc.vector.tensor_mul(
                                    exj, exj, mask_sb[:, t, off:512]
                                )
                            h = 2 * mt + j
                            nc.tensor.matmul(
                                acc[j][0:65, off:512],
                                lhsT=v_sb[:, kt, h * 65 : h * 65 + 65],
                                rhs=exj,
                                start=(kt == 0),
                                stop=(kt == nkt - 1),
                            )
                    for j in range(2):
                        dn = small.tile([1, 512], f32, tag="dn", name=f"dn{j}")
                        # reciprocal_approx_fast misreads PSUM on HW; bounce
                        # the denominator row through SBUF first.
                        nc.vector.tensor_copy(dn, acc[j][64:65, :])
                        rc = small.tile([1, 512], f32, tag="rc", name=f"rc{j}")
                        nc.vector.reciprocal_approx_fast(out=rc, in_=dn)
                        bc = small.tile([64, 512], f32, tag="bc", name=f"bc{j}")
                        nc.gpsimd.partition_broadcast(out_ap=bc, in_ap=rc)
                        pb = j * 64
                        nc.vector.tensor_mul(
                            at_sb[pb : pb + 64, mt, qc * 512 : qc * 512 + 512],
                            acc[j][0:64, :],
                            bc,
                        )

                # ---- out projection for this quarter ----
                for qt in range(qc * 4, qc * 4 + 4):
                    for nh in range(2):
                        ps = ps_acc.tile(
                            [128, 512], f32, tag="acc", name=f"ps_y{qt}{nh}"
                        )
                        for kt2 in range(2):
                            nc.tensor.matmul(
                                ps,
                                lhsT=at_sb[:, kt2, qt * 128 : qt * 128 + 128],
                                rhs=wo_sb[:, kt2, nh * 512 : nh * 512 + 512],
                                start=(kt2 == 0),
                                stop=(kt2 == 1),
                            )
                        nc.vector.tensor_copy(
                            y_sb[:, qt, nh * 512 : nh * 512 + 512], ps
                        )
                nc.sync.dma_start(
                    out=y[:, qc * 4 * E : (qc + 1) * 4 * E],
                    in_=y_sb[:, qc * 4 : (qc + 1) * 4, :],
                )

    nc.compile()
    return nc


def _get_program():
    global _prog
    if _prog is None:
        _prog = _build_program()
    return _prog


def _make_mask():
    import ml_dtypes

    k = np.arange(128)[:, None]
    q = np.arange(512)[None, :]
    m = np.stack([(q >= k + 128 * t) for t in range(4)])  # [4, 128, 512]
    return np.ascontiguousarray(
        m.transpose(1, 0, 2).reshape(128, 4 * 512)
    ).astype(ml_dtypes.bfloat16)


def _pack_rows(a, ktiles):
    """[ktiles*128, C] -> [128, ktiles*C] with per-partition contiguous rows."""
    kt, c = ktiles, a.shape[1]
    return np.ascontiguousarray(
        a.reshape(kt, 128, c).transpose(1, 0, 2).reshape(128, kt * c)
    )


def _core_inputs(x, Wq, bq, Wk, bk, Wv, bv, Wo, mask, c):
    import ml_dtypes

    bf16 = ml_dtypes.bfloat16
    b, g = divmod(c, 4)
    sl = slice(g * HD, (g + 1) * HD)
    xT = x[b].T  # [E, S]
    xT_p = np.ascontiguousarray(
        xT.reshape(KT, 128, QC, 512).transpose(1, 2, 0, 3).reshape(128, QC * KT * 512)
    )
    return {
        "xT": xT_p.astype(bf16),
        "wq": _pack_rows(Wq[:, sl], KT).astype(bf16),
        "wk": _pack_rows(Wk[:, sl], KT).astype(bf16),
        "wv": _pack_rows(Wv[:, sl], KT).astype(bf16),
        "wo": _pack_rows(Wo[sl, :], 2).astype(bf16),
        "bqc": np.ascontiguousarray(bq[sl].reshape(2, 128).T).astype(np.float32),
        "bkc": np.ascontiguousarray(bk[sl].reshape(2, 128).T).astype(np.float32),
        "bvb": np.ascontiguousarray(
            np.broadcast_to(bv[sl], (128, HD))
        ).astype(bf16),
        "mask": mask,
    }


def _unpack_y(y_p):
    """[128, NQT*E] -> [S, E]"""
    return y_p.reshape(128, NQT, E).transpose(1, 0, 2).reshape(S, E)


def kernel(x, Wq, bq, Wk, bk, Wv, bv, Wo, bo, **_run_kwargs):
    from concourse.bass_utils import run_bass_kernel_spmd

    x = np.asarray(x, dtype=np.float32)
    Wq, bq = np.asarray(Wq, np.float32), np.asarray(bq, np.float32)
    Wk, bk = np.asarray(Wk, np.float32), np.asarray(bk, np.float32)
    Wv, bv = np.asarray(Wv, np.float32), np.asarray(bv, np.float32)
    Wo, bo = np.asarray(Wo, np.float32), np.asarray(bo, np.float32)

    nc = _get_program()
    mask = _make_mask()
    in_maps = [
        _core_inputs(x, Wq, bq, Wk, bk, Wv, bv, Wo, mask, c) for c in range(NCORES)
    ]
    res = run_bass_kernel_spmd(nc, in_maps, list(range(NCORES)), **_run_kwargs)
    global LAST_RESULTS
    LAST_RESULTS = res
    parts = [_unpack_y(res.results[c]["y"]) for c in range(NCORES)]
    out = np.empty((B, S, E), np.float32)
    for b in range(B):
        out[b] = parts[4 * b] + parts[4 * b + 1] + parts[4 * b + 2] + parts[4 * b + 3]
        out[b] += bo
    return out



# revision 17
# speedup vs baseline: 1.1990x; 1.1990x over previous
"""Causal self-attention (B=2, S=2048, E=1024, H=16, D=64) on 8 TRN2 cores.

Sharding: core c = (batch b = c//4, head-group g = c%4) owns batch b and
heads 4g..4g+3 (a 256-wide slice of the QKV projections / Wo rows).
Each core computes its partial out-projection y_c = attout_c @ Wo_c; the
host sums the 4 partials per batch and adds bo (the tensor-parallel
out-proj all-reduce, done on host since cores are independent).

Device pipeline (per core), bf16 operands / fp32 PSUM accumulation:
  Q^T, K^T [256, S] via lhsT=W chunk, rhs=xT chunk
  V natural [S, 4*(64+1)] with a ones column per head
  scores^T [k, q] per head pair (PE row-groups 0-63/64-127), exp on ACT
  (scale=1/8 folded), causal mask multiply on DVE (diagonal 128x128
  blocks only)
  PV in natural orientation: acc[q, 65] += exp_slice^T @ (V|ones) --
  n=65 per matmul instead of n=512, halving PE time; column 64 is the
  softmax denominator per q-row, so normalization is a per-partition
  reciprocal + tensor_scalar multiply (no partition broadcast).
  normalized attout [q, 64] is transposed back to [chan, q] on the PE
  (weight-load-free transposes through a bf16 PSUM tile), then
  y = attout^T.T @ Wo staged in SBUF as bf16 and DMA'd per q-chunk.

The PE instruction stream is software-pipelined: the attention inner
loop is ACT(exp)-bound, so projection / transpose / out-projection
matmuls are interleaved between attention steps via a filler queue, and
the out-projection of chunk qc is deferred until after attention of
qc+1 so the PE never waits on the normalize chain.
"""

import numpy as np

B, S, E, H = 2, 2048, 1024, 16
D = E // H          # 64
NCORES = 8
HPC = 4             # heads per core
HD = HPC * D        # 256 cols per core
KT = E // 128       # 8 contraction tiles for projections
QC = S // 512       # 4 query chunks
NQT = S // 128      # 16 row tiles
VW = HPC * (D + 1)  # 260: V + ones column per head

_prog = None
LAST_RESULTS = None


def _build_program():
    import concourse.mybir as mybir
    import concourse.tile as tile
    from concourse import bacc

    f32 = mybir.dt.float32
    bf16 = mybir.dt.bfloat16
    Exp = mybir.ActivationFunctionType.Exp
    Identity = mybir.ActivationFunctionType.Identity

    nc = bacc.Bacc(trn_type="TRN2", target_bir_lowering=False, debug=False)

    xT = nc.dram_tensor("xT", [128, QC * KT * 512], bf16, kind="ExternalInput").ap()
    wq = nc.dram_tensor("wq", [128, KT * HD], bf16, kind="ExternalInput").ap()
    wk = nc.dram_tensor("wk", [128, KT * HD], bf16, kind="ExternalInput").ap()
    wv = nc.dram_tensor("wv", [128, KT * HD], bf16, kind="ExternalInput").ap()
    wo = nc.dram_tensor("wo", [128, 2 * E], bf16, kind="ExternalInput").ap()
    bq = nc.dram_tensor("bqc", [128, 2], f32, kind="ExternalInput").ap()
    bk = nc.dram_tensor("bkc", [128, 2], f32, kind="ExternalInput").ap()
    bv = nc.dram_tensor("bvb", [128, HD], bf16, kind="ExternalInput").ap()
    # cols 0-127: within-block causal mask (q >= k); cols 128-255: identity
    mi = nc.dram_tensor("maskid", [128, 256], bf16, kind="ExternalInput").ap()
    y = nc.dram_tensor("y", [128, NQT * E], bf16, kind="ExternalOutput").ap()

    with tile.TileContext(nc) as tc:
        with (
            tc.tile_pool(name="consts", bufs=1) as consts,
            tc.tile_pool(name="exps", bufs=4) as exps,
            tc.tile_pool(name="small", bufs=4) as small,
            tc.tile_pool(name="ps_sc", bufs=2, space="PSUM") as ps_sc,
            tc.tile_pool(name="ps_acc", bufs=2, space="PSUM") as ps_acc,
            tc.tile_pool(name="ps_out", bufs=2, space="PSUM") as ps_out,
        ):
            # ---- constants; DMA order tuned so qc=0 work starts ASAP ----
            xt_sb = consts.tile([128, QC, KT, 512], bf16)
            wq_sb = consts.tile([128, KT, HD], bf16)
            wk_sb = consts.tile([128, KT, HD], bf16)
            wv_sb = consts.tile([128, KT, HD], bf16)
            wo_sb = consts.tile([128, 2, E], bf16)
            mi_sb = consts.tile([128, 2, 128], bf16)
            bq_sb = consts.tile([128, 2], f32)
            bk_sb = consts.tile([128, 2], f32)
            bv_sb = consts.tile([128, HD], bf16)

            def load_xt(qc, k0, k1):
                nc.sync.dma_start(
                    out=xt_sb[:, qc, k0:k1],
                    in_=xT[:, (qc * KT + k0) * 512 : (qc * KT + k1) * 512].rearrange(
                        "p (kt c) -> p kt c", kt=k1 - k0
                    ),
                )

            # first Q-proj contraction tiles arrive first
            nc.sync.dma_start(
                out=wq_sb[:, 0:2],
                in_=wq[:, 0 : 2 * HD].rearrange("p (kt c) -> p kt c", kt=2),
            )
            load_xt(0, 0, 2)
            nc.sync.dma_start(
                out=wq_sb[:, 2:KT],
                in_=wq[:, 2 * HD :].rearrange("p (kt c) -> p kt c", kt=KT - 2),
            )
            load_xt(0, 2, KT)
            nc.sync.dma_start(out=wk_sb, in_=wk.rearrange("p (kt c) -> p kt c", kt=KT))
            nc.sync.dma_start(out=wv_sb, in_=wv.rearrange("p (kt c) -> p kt c", kt=KT))
            nc.sync.dma_start(out=bq_sb, in_=bq)
            nc.sync.dma_start(out=bk_sb, in_=bk)
            nc.sync.dma_start(out=bv_sb, in_=bv)
            nc.sync.dma_start(out=mi_sb, in_=mi.rearrange("p (t c) -> p t c", t=2))
            load_xt(1, 0, KT)
            load_xt(2, 0, KT)
            load_xt(3, 0, KT)
            nc.sync.dma_start(out=wo_sb, in_=wo.rearrange("p (kt c) -> p kt c", kt=2))

            mask_sb = mi_sb[:, 0]
            ident_sb = mi_sb[:, 1]

            # ---- persistent activations ----
            # Q^T/K^T: [128, mt, S]; mt=0 holds cols 0-127 (heads 0,1),
            # mt=1 holds cols 128-255 (heads 2,3).
            qt_sb = consts.tile([128, 2, S], bf16)
            kt_sb = consts.tile([128, 2, S], bf16)
            # V natural: [row-in-tile, rt, 4*(64+1)]; per head h cols
            # h*65..h*65+63 are V, col h*65+64 is ones (softmax denom).
            v_sb = consts.tile([128, NQT, VW], bf16)
            nc.vector.memset(
                v_sb.rearrange("p rt (h c) -> p rt h c", h=HPC)[:, :, :, D : D + 1],
                1.0,
            )
            # normalized attout^T, same layout as qt_sb
            at_sb = consts.tile([128, 2, S], bf16)
            # bf16 output staging
            y_sb = consts.tile([128, NQT, E], bf16)

            # ---------- emitters ----------
            # projections are emitted in kt-halves (~0.4us of PE work each)
            # so they can act as fine-grained fillers in the filler queue.
            # projection PSUM comes from the ps_out pool (1-bank tiles) so
            # proj fillers never contend with the scores/exp pipeline slots.
            def qk_proj_closures(qc, dst_idx, mt):
                w_sb, b_sb, dst = (
                    (wq_sb, bq_sb, qt_sb),
                    (wk_sb, bk_sb, kt_sb),
                )[dst_idx]
                ps = [None]

                def part(k0, k1):
                    if ps[0] is None:
                        ps[0] = ps_out.tile(
                            [128, 512], f32, tag="out",
                            name=f"ps_qk{qc}{dst_idx}{mt}",
                        )
                    o = ps[0]
                    for kt in range(k0, k1):
                        nc.tensor.matmul(
                            o,
                            lhsT=w_sb[:, kt, mt * 128 : mt * 128 + 128],
                            rhs=xt_sb[:, qc, kt],
                            start=(kt == 0),
                            stop=(kt == KT - 1),
                        )
                    if k1 == KT:
                        nc.vector.tensor_scalar_add(
                            dst[:, mt, qc * 512 : (qc + 1) * 512],
                            o,
                            b_sb[:, mt : mt + 1],
                        )

                return [
                    lambda a=a, b=b: part(a, b) for a, b in ((0, 2), (2, 5), (5, KT))
                ]

            def v_proj_closures(qc, rl):
                # one row-tile per PSUM bank: a second accumulation group in
                # the same bank would re-zero it (start=True zeroes 2KB)
                ps = [None]

                def part(k0, k1):
                    if ps[0] is None:
                        ps[0] = ps_out.tile(
                            [128, 256], f32, tag="out", name=f"ps_v{qc}{rl}"
                        )
                    rt = qc * 4 + rl
                    o = ps[0]
                    for kt in range(k0, k1):
                        nc.tensor.matmul(
                            o,
                            lhsT=xt_sb[:, qc, kt, rl * 128 : rl * 128 + 128],
                            rhs=wv_sb[:, kt, :],
                            start=(kt == 0),
                            stop=(kt == KT - 1),
                        )
                    if k1 == KT:
                        nc.vector.tensor_add(
                            v_sb[:, rt, :].rearrange("p (h c) -> p h c", h=HPC)[
                                :, :, 0:D
                            ],
                            o.rearrange("p (h c) -> p h c", h=HPC),
                            bv_sb.rearrange("p (h c) -> p h c", h=HPC),
                        )

                return [lambda a=a, b=b: part(a, b) for a, b in ((0, 4), (4, KT))]

            def emit_attn(qc, mt, pump):
                """scores -> exp -> mask -> natural-PV for one head pair.

                Returns the two accumulation tiles [128, 4*65] (one per
                head j); col qt*65+64 is the softmax denominator.
                pump(n) emits up to n queued filler closures between
                attention steps to keep the PE busy while ACT runs exp.
                """
                nkt = 4 * (qc + 1)
                acc = [
                    ps_acc.tile(
                        [128, 260], f32, tag="acc", name=f"acc{qc}{mt}{j}"
                    )
                    for j in range(2)
                ]
                exv = [None] * nkt

                def emit_pv(kt):
                    # PSUM start=True zeroes the whole 2KB bank, so each acc
                    # bank gets exactly one start (first matmul at kt=0) and
                    # one stop (last matmul, which is qt=3 at kt=nkt-1); the
                    # per-qt column groups accumulate independently by
                    # address in between.
                    t = kt - 4 * qc
                    ex = exv[kt]
                    exv[kt] = None
                    for j in range(2):
                        h = 2 * mt + j
                        for qt in range(3, max(t, 0) - 1, -1):
                            nc.tensor.matmul(
                                acc[j][:, qt * 65 : qt * 65 + 65],
                                lhsT=ex[:, j * 512 + qt * 128 : j * 512 + (qt + 1) * 128],
                                rhs=v_sb[:, kt, h * 65 : h * 65 + 65],
                                start=(kt == 0 and qt == 3),
                                stop=(kt == nkt - 1),
                                skip_group_check=True,
                            )

                for kt in range(nkt):
                    t = kt - 4 * qc
                    off = 128 * t if t > 0 else 0
                    ps = ps_sc.tile([128, 1024], f32, tag="sc", name=f"ps_s{qc}{mt}{kt}")
                    for j in range(2):
                        pb = j * 64
                        nc.tensor.matmul(
                            ps[:, j * 512 + off : j * 512 + 512],
                            lhsT=kt_sb[pb : pb + 64, mt, kt * 128 : kt * 128 + 128],
                            rhs=qt_sb[
                                pb : pb + 64, mt, qc * 512 + off : qc * 512 + 512
                            ],
                            start=True,
                            stop=True,
                        )
                    ex = exps.tile([128, 1024], bf16, tag="ex", name=f"ex{qc}{mt}{kt}")
                    exv[kt] = ex
                    # scores scale 1/sqrt(D) folded into exp
                    if off == 0:
                        nc.scalar.activation(ex, ps, Exp, scale=0.125)
                    else:
                        for j in range(2):
                            nc.scalar.activation(
                                ex[:, j * 512 + off : j * 512 + 512],
                                ps[:, j * 512 + off : j * 512 + 512],
                                Exp,
                                scale=0.125,
                            )
                    if t >= 0:  # mask the diagonal 128x128 block only
                        for j in range(2):
                            blk = ex[:, j * 512 + t * 128 : j * 512 + (t + 1) * 128]
                            nc.vector.tensor_mul(blk, blk, mask_sb)
                    # PV lags the QK/exp pipeline by 2 steps so the PE never
                    # arrives before the exp of that step has drained.
                    pump(1)
                    if kt >= 2:
                        emit_pv(kt - 2)
                    pump(1)
                if nkt >= 2:
                    pump(1)
                    emit_pv(nkt - 2)
                pump(2)
                emit_pv(nkt - 1)
                return acc

            def emit_norm(qc, mt, acc):
                """DVE-only: divide attout rows by the denominator column."""
                an = small.tile(
                    [128, 4, 2, D], bf16, tag="an", name=f"an{qc}{mt}", bufs=2
                )
                for j in range(2):
                    accr = acc[j].rearrange("p (qt c) -> p qt c", qt=4)
                    dnb = small.tile([128, 4], f32, tag="dn", name=f"dn{qc}{mt}{j}")
                    nc.vector.tensor_copy(dnb, accr[:, :, D])
                    rc = small.tile([128, 4], f32, tag="rc", name=f"rc{qc}{mt}{j}")
                    nc.vector.reciprocal(rc, dnb)
                    for qt in range(4):
                        nc.vector.tensor_scalar_mul(
                            an[:, qt, j], accr[:, qt, 0:D], rc[:, qt : qt + 1]
                        )
                return an

            def emit_transp(qc, mt, an):
                """PE transposes [q128, 128ch] -> [128ch, q128], one per
                q-tile covering both heads, then one DVE copy into at_sb."""
                atT = ps_out.tile([128, 512], bf16, tag="out", name=f"atT{qc}{mt}")
                for qt in range(4):
                    # single start/stop lifecycle for the shared bank
                    nc.tensor.matmul(
                        atT[:, qt * 128 : (qt + 1) * 128],
                        lhsT=an[:, qt].rearrange("p j c -> p (j c)"),
                        rhs=ident_sb,
                        is_transpose=True,
                        start=(qt == 0),
                        stop=(qt == 3),
                        skip_group_check=True,
                    )
                nc.vector.tensor_copy(at_sb[:, mt, qc * 512 : (qc + 1) * 512], atT)

            def emit_outproj_qt(qt, nh):
                ps = ps_out.tile([128, 512], f32, tag="out", name=f"ps_y{qt}{nh}")
                for kt2 in range(2):
                    nc.tensor.matmul(
                        ps,
                        lhsT=at_sb[:, kt2, qt * 128 : qt * 128 + 128],
                        rhs=wo_sb[:, kt2, nh * 512 : nh * 512 + 512],
                        start=(kt2 == 0),
                        stop=(kt2 == 1),
                    )
                dst = y_sb[:, qt, nh * 512 : nh * 512 + 512]
                # GPSIMD cannot read PSUM on hardware; split between DVE/ACT
                if nh == 0:
                    nc.vector.tensor_copy(dst, ps)
                else:
                    nc.scalar.activation(dst, ps, Identity)

            def emit_y_dma(qc):
                if qc < QC - 1:
                    nc.sync.dma_start(
                        out=y[:, qc * 4 * E : (qc + 1) * 4 * E],
                        in_=y_sb[:, qc * 4 : (qc + 1) * 4, :],
                    )
                else:  # split the last chunk per row-tile to shorten the tail
                    for qt in range(qc * 4, qc * 4 + 4):
                        nc.sync.dma_start(
                            out=y[:, qt * E : (qt + 1) * E],
                            in_=y_sb[:, qt : qt + 1, :],
                        )

            # ---------- main schedule ----------
            from collections import deque

            fillers = deque()

            def pump(n):
                for _ in range(n):
                    if not fillers:
                        return
                    fillers.popleft()()

            def drain():
                while fillers:
                    fillers.popleft()()

            def queue_proj(qc):
                for mt in range(2):
                    fillers.extend(qk_proj_closures(qc, 0, mt))
                    fillers.extend(qk_proj_closures(qc, 1, mt))
                for rl in range(4):
                    fillers.extend(v_proj_closures(qc, rl))

            # qc=0 projections run before attention (DMA-paced startup)
            queue_proj(0)
            drain()

            def queue_outproj(pq):
                for qt in range(pq * 4, pq * 4 + 4):
                    for nh in range(2):
                        fillers.append(lambda t=qt, n=nh: emit_outproj_qt(t, n))
                fillers.append(lambda q=pq: emit_y_dma(q))

            # filler supply per attention phase, balanced against each
            # phase's ACT-bound slack (grows with qc): qc=3 has no next
            # projections, so all deferred out-projections are spent there.
            # proj and outproj fillers never overlap, so the ps_out pool
            # sees at most one projection plus the transpose staging tile.
            supply = {0: [], 1: [], 2: [], 3: [0, 1, 2]}

            for qc in range(QC):
                if qc + 1 < QC:
                    queue_proj(qc + 1)
                for pq in supply[qc]:
                    queue_outproj(pq)

                acc0 = emit_attn(qc, 0, pump)
                an0 = emit_norm(qc, 0, acc0)
                # mt=0 transposes run as a filler inside the mt=1 phase,
                # after the DVE normalize has had time to finish
                fillers.append(lambda q=qc, a=an0: emit_transp(q, 0, a))

                acc1 = emit_attn(qc, 1, pump)
                an1 = emit_norm(qc, 1, acc1)
                drain()
                emit_transp(qc, 1, an1)

            for qt in range(12, 16):
                for nh in range(2):
                    emit_outproj_qt(qt, nh)
            emit_y_dma(3)

    nc.compile()
    return nc


def _get_program():
    global _prog
    if _prog is None:
        _prog = _build_program()
    return _prog


def _make_maskid():
    import ml_dtypes

    k = np.arange(128)[:, None]
    q = np.arange(128)[None, :]
    m = (q >= k).astype(np.float32)          # [128, 128] causal block
    ident = np.eye(128, dtype=np.float32)
    return np.ascontiguousarray(
        np.concatenate([m, ident], axis=1)
    ).astype(ml_dtypes.bfloat16)


def _pack_rows(a, ktiles):
    """[ktiles*128, C] -> [128, ktiles*C] with per-partition contiguous rows."""
    kt, c = ktiles, a.shape[1]
    return np.ascontiguousarray(
        a.reshape(kt, 128, c).transpose(1, 0, 2).reshape(128, kt * c)
    )


def _core_inputs(x, Wq, bq, Wk, bk, Wv, bv, Wo, maskid, c):
    import ml_dtypes

    bf16 = ml_dtypes.bfloat16
    b, g = divmod(c, 4)
    sl = slice(g * HD, (g + 1) * HD)
    xT = x[b].T  # [E, S]
    xT_p = np.ascontiguousarray(
        xT.reshape(KT, 128, QC, 512).transpose(1, 2, 0, 3).reshape(128, QC * KT * 512)
    )
    return {
        "xT": xT_p.astype(bf16),
        "wq": _pack_rows(Wq[:, sl], KT).astype(bf16),
        "wk": _pack_rows(Wk[:, sl], KT).astype(bf16),
        "wv": _pack_rows(Wv[:, sl], KT).astype(bf16),
        "wo": _pack_rows(Wo[sl, :], 2).astype(bf16),
        "bqc": np.ascontiguousarray(bq[sl].reshape(2, 128).T).astype(np.float32),
        "bkc": np.ascontiguousarray(bk[sl].reshape(2, 128).T).astype(np.float32),
        "bvb": np.ascontiguousarray(
            np.broadcast_to(bv[sl], (128, HD))
        ).astype(bf16),
        "maskid": maskid,
    }


def _unpack_y(y_p):
    """[128, NQT*E] bf16 -> [S, E] f32"""
    return (
        y_p.astype(np.float32).reshape(128, NQT, E).transpose(1, 0, 2).reshape(S, E)
    )


def kernel(x, Wq, bq, Wk, bk, Wv, bv, Wo, bo, **_run_kwargs):
    from concourse.bass_utils import run_bass_kernel_spmd

    x = np.asarray(x, dtype=np.float32)
    Wq, bq = np.asarray(Wq, np.float32), np.asarray(bq, np.float32)
    Wk, bk = np.asarray(Wk, np.float32), np.asarray(bk, np.float32)
    Wv, bv = np.asarray(Wv, np.float32), np.asarray(bv, np.float32)
    Wo, bo = np.asarray(Wo, np.float32), np.asarray(bo, np.float32)

    nc = _get_program()
    maskid = _make_maskid()
    in_maps = [
        _core_inputs(x, Wq, bq, Wk, bk, Wv, bv, Wo, maskid, c) for c in range(NCORES)
    ]
    res = run_bass_kernel_spmd(nc, in_maps, list(range(NCORES)), **_run_kwargs)
    global LAST_RESULTS
    LAST_RESULTS = res
    parts = [_unpack_y(res.results[c]["y"]) for c in range(NCORES)]
    out = np.empty((B, S, E), np.float32)
    for b in range(B):
        out[b] = parts[4 * b] + parts[4 * b + 1] + parts[4 * b + 2] + parts[4 * b + 3]
        out[b] += bo
    return out
